# revision 1
# baseline (speedup 1.0000x reference)
"""Guided channel-wise 3x3 conv (per-pixel weights) on 8 Trainium2 cores.

out[b,c,h,w] = sum_{dh,dw in {-1,0,1}} input[b,c,h+dh,w+dw] * weights[b,c,k(dh,dw),h,w]
with SAME zero padding.  Shapes: input (8,64,128,128) f32,
weights (8,64,9,128,128) f32 -> out (8,64,128,128) f32.

Sharding: pure data parallelism, one batch sample per NeuronCore (B=8 cores).

Per-core layout: 128 SBUF partitions = (half, c) with p = half*64 + c; each
partition holds one 64-row half of one channel plane.  The input is pre-padded
on the host into the exact per-partition SBUF layout (66 padded rows x 130
padded cols, zeros on border/halo) and the weights are pre-transposed to
(9, 128, 64*128), so every SBUF tensor is filled by one large contiguous DMA.

Raw bass (no Tile): the walrus build in this container only allows ONE sync
wait per instruction, so all synchronization is explicit standalone wait_ge
instructions + then_inc completions.  SP streams the 9 tap-weight DMAs through
two double-buffered slots while DVE runs mult+accumulate per tap.
"""

import numpy as np

from concourse import bass, mybir
from concourse.bass_utils import run_bass_kernel_spmd

B, CI, H, W = 8, 64, 128, 128
K = 9
HH = H // 2  # rows per half-plane (64)
PR = HH + 2  # padded rows per partition (66)
PC = W + 2  # padded cols (130)
NP = 128  # SBUF partitions
FP = HH * W  # free elems per partition of one output half-plane (8192)

F32 = mybir.dt.float32

TAPS = [4, 0, 1, 2, 3, 5, 6, 7, 8]  # center tap first: it initializes out


def build_bass():
    nc = bass.Bass()
    inp = nc.declare_dram_parameter("input", [NP, PR * PC], F32, isOutput=False)
    wts = nc.declare_dram_parameter("weights", [K, NP, FP], F32, isOutput=False)
    out = nc.declare_dram_parameter("out", [NP, FP], F32, isOutput=True)

    from contextlib import ExitStack

    with ExitStack() as ctx:
        in_pad = ctx.enter_context(nc.sbuf_tensor("in_pad", [NP, PR * PC], F32))
        wt0 = ctx.enter_context(nc.sbuf_tensor("wt0", [NP, FP], F32))
        wt1 = ctx.enter_context(nc.sbuf_tensor("wt1", [NP, FP], F32))
        tmp = ctx.enter_context(nc.sbuf_tensor("tmp", [NP, FP], F32))
        out_t = ctx.enter_context(nc.sbuf_tensor("out_t", [NP, FP], F32))
        block = ctx.enter_context(nc.Block())
        dma_sem = ctx.enter_context(nc.semaphore("dma_sem"))
        dve_sem = ctx.enter_context(nc.semaphore("dve_sem"))

        wt_slots = (wt0, wt1)
        in3 = in_pad[:].rearrange("p (r w) -> p r w", r=PR)
        out3 = out_t[:].rearrange("p (r w) -> p r w", r=HH)
        tmp3 = tmp[:].rearrange("p (r w) -> p r w", r=HH)

        RH = HH // 2  # 32-row split for ramp-up/drain overlap
        HF = FP // 2  # free elems of a 32-row block (4096)

        @block.sync
        def _(sync):
            # Startup split: first mult half-block can start after ~half the
            # startup bytes have landed.
            sync.dma_start(out=in_pad[:, 0 : (RH + 2) * PC], in_=inp[:, 0 : (RH + 2) * PC]).then_inc(dma_sem, 16)
            sync.dma_start(out=wt_slots[0][:, 0:HF], in_=wts[TAPS[0], :, 0:HF]).then_inc(dma_sem, 16)
            sync.dma_start(out=in_pad[:, (RH + 2) * PC :], in_=inp[:, (RH + 2) * PC :]).then_inc(dma_sem, 16)
            sync.dma_start(out=wt_slots[0][:, HF:FP], in_=wts[TAPS[0], :, HF:FP]).then_inc(dma_sem, 16)
            for j, k in enumerate(TAPS):
                if j == 0:
                    continue
                if j >= 2:
                    # slot j%2 was last read by mult_{j-2}, done when dve_sem >= j
                    sync.wait_ge(dve_sem, j)
                sync.dma_start(out=wt_slots[j % 2][:], in_=wts[k]).then_inc(dma_sem, 16)
            # Drain split: flush the first half-block while the last add runs.
            sync.wait_ge(dve_sem, 11)
            sync.dma_start(out=out[:, 0:HF], in_=out_t[:, 0:HF]).then_inc(dma_sem, 16)
            sync.wait_ge(dve_sem, 12)
            sync.dma_start(out=out[:, HF:FP], in_=out_t[:, HF:FP]).then_inc(dma_sem, 16)
            sync.wait_ge(dma_sem, 16 * (K + 5))

        @block.vector
        def _(vector):
            for j, k in enumerate(TAPS):
                dh, dw = k // 3, k % 3
                wt3 = wt_slots[j % 2][:].rearrange("p (r w) -> p r w", r=HH)
                if j == 0:
                    # split into two 32-row multiplies for earlier start
                    vector.wait_ge(dma_sem, 32)  # in_a + wt0_a
                    vector.tensor_tensor(
                        out=out3[:, 0:RH],
                        in0=in3[:, dh : dh + RH, dw : dw + W],
                        in1=wt3[:, 0:RH],
                        op=mybir.AluOpType.mult,
                    ).then_inc(dve_sem, 1)
                    vector.wait_ge(dma_sem, 64)  # in_b + wt0_b
                    vector.tensor_tensor(
                        out=out3[:, RH:HH],
                        in0=in3[:, dh + RH : dh + HH, dw : dw + W],
                        in1=wt3[:, RH:HH],
                        op=mybir.AluOpType.mult,
                    ).then_inc(dve_sem, 1)
                    continue
                vector.wait_ge(dma_sem, 16 * (j + 4))  # startup 4 DMAs + taps 1..j
                iv = in3[:, dh : dh + HH, dw : dw + W]
                vector.tensor_tensor(
                    out=tmp3, in0=iv, in1=wt3, op=mybir.AluOpType.mult
                ).then_inc(dve_sem, 1)
                if j == len(TAPS) - 1:
                    # split the final accumulate so the first half can flush
                    vector.tensor_tensor(
                        out=out3[:, 0:RH],
                        in0=out3[:, 0:RH],
                        in1=tmp3[:, 0:RH],
                        op=mybir.AluOpType.add,
                    ).then_inc(dve_sem, 1)
                    vector.tensor_tensor(
                        out=out3[:, RH:HH],
                        in0=out3[:, RH:HH],
                        in1=tmp3[:, RH:HH],
                        op=mybir.AluOpType.add,
                    ).then_inc(dve_sem, 1)
                else:
                    vector.tensor_tensor(
                        out=out3, in0=out3, in1=tmp3, op=mybir.AluOpType.add
                    )

    return nc


def _prep_input(x):
    """(64,128,128) -> (128, 66*130) per-partition padded layout."""
    pad = np.zeros((CI, H + 2, W + 2), dtype=np.float32)
    pad[:, 1 : H + 1, 1 : W + 1] = x
    win = np.stack([pad[:, 0:PR, :], pad[:, HH : HH + PR, :]], axis=0)  # (2,64,66,130)
    return np.ascontiguousarray(win.reshape(NP, PR * PC))


def _prep_weights(w):
    """(64,9,128,128) -> (9, 128, 64*128) with partition p = half*64 + c."""
    wr = w.reshape(CI, K, 2, HH, W).transpose(1, 2, 0, 3, 4)  # (9,2,64,64,128)
    return np.ascontiguousarray(wr.reshape(K, NP, FP))


def _unprep_out(o):
    """(128, 64*128) -> (64,128,128)."""
    return np.ascontiguousarray(
        o.reshape(2, CI, HH, W).transpose(1, 0, 2, 3).reshape(CI, H, W)
    )


_NC = None


def _get_nc():
    global _NC
    if _NC is None:
        _NC = build_bass()
    return _NC


def make_in_maps(input, weights):
    input = np.asarray(input, dtype=np.float32)
    weights = np.asarray(weights, dtype=np.float32)
    return [
        {"input": _prep_input(input[b]), "weights": _prep_weights(weights[b])}
        for b in range(B)
    ]


def kernel(input, weights):
    nc = _get_nc()
    in_maps = make_in_maps(input, weights)
    res = run_bass_kernel_spmd(nc, in_maps, list(range(B)))
    return np.stack([_unprep_out(res.results[b]["out"]) for b in range(B)], axis=0)



# revision 3
# speedup vs baseline: 1.8286x; 1.8286x over previous
"""Guided channel-wise 3x3 conv (per-pixel weights) on 8 Trainium2 cores.

out[b,c,h,w] = sum_{dh,dw in {-1,0,1}} input[b,c,h+dh,w+dw] * weights[b,c,k(dh,dw),h,w]
with SAME zero padding.  Shapes: input (8,64,128,128) f32,
weights (8,64,9,128,128) f32 -> out (8,64,128,128) f32.

Sharding: pure data parallelism, one batch sample per NeuronCore (B=8 cores).

Per-core layout: 128 SBUF partitions = (half, c) with p = half*64 + c; each
partition holds one 64-row half of one channel plane.  The input is pre-padded
on the host into the exact per-partition SBUF layout (66 padded rows x 130
padded cols, zeros on border/halo) and the weights are pre-transposed to
(9, 128, 64*128), so every SBUF tensor is filled by one large contiguous DMA.

All on-chip data is fp16: the 2e-2 rel-err gate leaves plenty of headroom and
fp16 halves both the HBM traffic (memory-bound regime) and the DVE time
(2x_1p fast mode needs 2-byte packed operands).

Raw bass (no Tile): the walrus build in this container only allows ONE sync
wait per instruction, so all synchronization is explicit standalone wait_ge
instructions + then_inc completions.  SP streams the 9 tap-weight DMAs through
three buffered slots while DVE runs mult+accumulate per tap.
"""

import numpy as np

from concourse import bass, mybir
from concourse.bass_utils import run_bass_kernel_spmd

B, CI, H, W = 8, 64, 128, 128
K = 9
HH = H // 2  # rows per half-plane (64)
PR = HH + 2  # padded rows per partition (66)
PC = W + 2  # padded cols (130)
NP = 128  # SBUF partitions
FP = HH * W  # free elems per partition of one output half-plane (8192)

F16 = mybir.dt.float16

TAPS = [4, 0, 1, 2, 3, 5, 6, 7, 8]  # center tap first: it initializes out
NSLOT = 3


def build_bass():
    nc = bass.Bass()
    inp = nc.declare_dram_parameter("input", [NP, PR * PC], F16, isOutput=False)
    wts = nc.declare_dram_parameter("weights", [K, NP, FP], F16, isOutput=False)
    out = nc.declare_dram_parameter("out", [NP, FP], F16, isOutput=True)

    from contextlib import ExitStack

    with ExitStack() as ctx:
        in_pad = ctx.enter_context(nc.sbuf_tensor("in_pad", [NP, PR * PC], F16))
        wt_slots = [
            ctx.enter_context(nc.sbuf_tensor(f"wt{i}", [NP, FP], F16))
            for i in range(NSLOT)
        ]
        tmp = ctx.enter_context(nc.sbuf_tensor("tmp", [NP, FP], F16))
        out_t = ctx.enter_context(nc.sbuf_tensor("out_t", [NP, FP], F16))
        block = ctx.enter_context(nc.Block())
        dma_sem = ctx.enter_context(nc.semaphore("dma_sem"))
        dve_sem = ctx.enter_context(nc.semaphore("dve_sem"))

        in3 = in_pad[:].rearrange("p (r w) -> p r w", r=PR)
        out3 = out_t[:].rearrange("p (r w) -> p r w", r=HH)
        tmp3 = tmp[:].rearrange("p (r w) -> p r w", r=HH)

        RH = HH // 2  # 32-row split for ramp-up/drain overlap
        HF = FP // 2  # free elems of a 32-row block (4096)

        @block.sync
        def _(sync):
            # Startup split: first mult half-block can start after ~half the
            # startup bytes have landed.
            sync.dma_start(out=in_pad[:, 0 : (RH + 2) * PC], in_=inp[:, 0 : (RH + 2) * PC]).then_inc(dma_sem, 16)
            sync.dma_start(out=wt_slots[0][:, 0:HF], in_=wts[TAPS[0], :, 0:HF]).then_inc(dma_sem, 16)
            sync.dma_start(out=in_pad[:, (RH + 2) * PC :], in_=inp[:, (RH + 2) * PC :]).then_inc(dma_sem, 16)
            sync.dma_start(out=wt_slots[0][:, HF:FP], in_=wts[TAPS[0], :, HF:FP]).then_inc(dma_sem, 16)
            for j, k in enumerate(TAPS):
                if j == 0:
                    continue
                if j >= NSLOT:
                    # slot j%NSLOT was last read by mult_{j-NSLOT}; tap0 incs
                    # dve_sem twice, taps 1.. inc once -> done at 2+(j-NSLOT)
                    sync.wait_ge(dve_sem, j - NSLOT + 2)
                sync.dma_start(out=wt_slots[j % NSLOT][:], in_=wts[k]).then_inc(dma_sem, 16)
            # Drain split: flush the first half-block while the last add runs.
            sync.wait_ge(dve_sem, 11)
            sync.dma_start(out=out[:, 0:HF], in_=out_t[:, 0:HF]).then_inc(dma_sem, 16)
            sync.wait_ge(dve_sem, 12)
            sync.dma_start(out=out[:, HF:FP], in_=out_t[:, HF:FP]).then_inc(dma_sem, 16)
            sync.wait_ge(dma_sem, 16 * (K + 5))

        @block.vector
        def _(vector):
            for j, k in enumerate(TAPS):
                dh, dw = k // 3, k % 3
                wt3 = wt_slots[j % NSLOT][:].rearrange("p (r w) -> p r w", r=HH)
                if j == 0:
                    # split into two 32-row multiplies for earlier start
                    vector.wait_ge(dma_sem, 32)  # in_a + wt0_a
                    vector.tensor_tensor(
                        out=out3[:, 0:RH],
                        in0=in3[:, dh : dh + RH, dw : dw + W],
                        in1=wt3[:, 0:RH],
                        op=mybir.AluOpType.mult,
                    ).then_inc(dve_sem, 1)
                    vector.wait_ge(dma_sem, 64)  # in_b + wt0_b
                    vector.tensor_tensor(
                        out=out3[:, RH:HH],
                        in0=in3[:, dh + RH : dh + HH, dw : dw + W],
                        in1=wt3[:, RH:HH],
                        op=mybir.AluOpType.mult,
                    ).then_inc(dve_sem, 1)
                    continue
                vector.wait_ge(dma_sem, 16 * (j + 4))  # startup 4 DMAs + taps 1..j
                iv = in3[:, dh : dh + HH, dw : dw + W]
                vector.tensor_tensor(
                    out=tmp3, in0=iv, in1=wt3, op=mybir.AluOpType.mult
                ).then_inc(dve_sem, 1)
                if j == len(TAPS) - 1:
                    # split the final accumulate so the first half can flush
                    vector.tensor_tensor(
                        out=out3[:, 0:RH],
                        in0=out3[:, 0:RH],
                        in1=tmp3[:, 0:RH],
                        op=mybir.AluOpType.add,
                    ).then_inc(dve_sem, 1)
                    vector.tensor_tensor(
                        out=out3[:, RH:HH],
                        in0=out3[:, RH:HH],
                        in1=tmp3[:, RH:HH],
                        op=mybir.AluOpType.add,
                    ).then_inc(dve_sem, 1)
                else:
                    vector.tensor_tensor(
                        out=out3, in0=out3, in1=tmp3, op=mybir.AluOpType.add
                    )

    return nc


def _prep_input(x):
    """(64,128,128) f32 -> (128, 66*130) fp16 per-partition padded layout."""
    pad = np.zeros((CI, H + 2, W + 2), dtype=np.float16)
    pad[:, 1 : H + 1, 1 : W + 1] = x.astype(np.float16)
    win = np.stack([pad[:, 0:PR, :], pad[:, HH : HH + PR, :]], axis=0)  # (2,64,66,130)
    return np.ascontiguousarray(win.reshape(NP, PR * PC))


def _prep_weights(w):
    """(64,9,128,128) f32 -> (9, 128, 64*128) fp16 with partition p = half*64 + c."""
    wr = w.astype(np.float16).reshape(CI, K, 2, HH, W).transpose(1, 2, 0, 3, 4)
    return np.ascontiguousarray(wr.reshape(K, NP, FP))


def _unprep_out(o):
    """(128, 64*128) fp16 -> (64,128,128) f32."""
    return np.ascontiguousarray(
        o.astype(np.float32).reshape(2, CI, HH, W).transpose(1, 0, 2, 3).reshape(CI, H, W)
    )


_NC = None


def _get_nc():
    global _NC
    if _NC is None:
        _NC = build_bass()
    return _NC


def make_in_maps(input, weights):
    input = np.asarray(input, dtype=np.float32)
    weights = np.asarray(weights, dtype=np.float32)
    return [
        {"input": _prep_input(input[b]), "weights": _prep_weights(weights[b])}
        for b in range(B)
    ]


def kernel(input, weights):
    nc = _get_nc()
    in_maps = make_in_maps(input, weights)
    res = run_bass_kernel_spmd(nc, in_maps, list(range(B)))
    return np.stack([_unprep_out(res.results[b]["out"]) for b in range(B)], axis=0)


# revision 9
# speedup vs baseline: 1.9113x; 1.0452x over previous
"""Guided channel-wise 3x3 conv (per-pixel weights) on 8 Trainium2 cores.

out[b,c,h,w] = sum_{dh,dw in {-1,0,1}} input[b,c,h+dh,w+dw] * weights[b,c,k(dh,dw),h,w]
with SAME zero padding.  Shapes: input (8,64,128,128) f32,
weights (8,64,9,128,128) f32 -> out (8,64,128,128) f32.

Sharding: pure data parallelism, one batch sample per NeuronCore (B=8 cores).

Per-core layout: 128 SBUF partitions = (half, c) with p = half*64 + c; each
partition holds one 64-row half of one channel plane (input padded 66x130).
Everything on-chip is fp16: halves HBM traffic (memory-bound regime) and
doubles DVE throughput (2x_1p mode needs 2-byte packed operands); rel err
~6e-4 vs the 2e-2 gate.

The DVE does 9 multiplies + 8 accumulates (~75 us of engine time); the
pipeline hides DMA behind it:
  - taps 0-2 are processed in 16-row quarters gated on quarter-granular
    weight DMAs, so compute starts as soon as the first ~1 MB lands and the
    serial fill (input + first taps) is overlapped,
  - taps 3-8 stream whole planes through 4 buffer slots (DMA runs ahead),
  - the final accumulate runs in quarters, each immediately flushed to HBM.

DMA completions are OUT OF ORDER on this hardware (queue packets fan out
over 16 DMA engines), so a single cumulative DMA semaphore is unsound.
Every awaited transfer group gets a private semaphore, and consumers wait
for that semaphore's full count — correct under any completion order.
"""

import numpy as np

from concourse import bass, mybir
from concourse.bass_utils import run_bass_kernel_spmd

B, CI, H, W = 8, 64, 128, 128
K = 9
HH = H // 2  # rows per half-plane (64)
PR = HH + 2  # padded rows per partition (66)
PC = W + 2  # padded cols (130)
NP = 128  # SBUF partitions
FP = HH * W  # free elems per partition of one output half-plane (8192)
QF = FP // 4  # quarter free elems (2048)
QR = HH // 4  # quarter rows (16)

F16 = mybir.dt.float16

TAPS = [4, 0, 1, 2, 3, 5, 6, 7, 8]  # center tap first: it initializes out
NSLOT = 4
NQTAP = 3  # taps processed in quarters (0..NQTAP-1)

# input DMA pieces: padded row ranges (disjoint); piece q covers the rows
# needed by quarter q of any tap (16q+dh .. 16q+16+dh, dh<=2)
IN_PIECES = [(0, 18), (18, 34), (34, 50), (50, 66)]


def build_bass():
    nc = bass.Bass()
    inp = nc.declare_dram_parameter("input", [NP, PR * PC], F16, isOutput=False)
    wts = nc.declare_dram_parameter("weights", [K, NP, FP], F16, isOutput=False)
    out = nc.declare_dram_parameter("out", [NP, FP], F16, isOutput=True)

    from contextlib import ExitStack

    with ExitStack() as ctx:
        in_pad = ctx.enter_context(nc.sbuf_tensor("in_pad", [NP, PR * PC], F16))
        wt = [
            ctx.enter_context(nc.sbuf_tensor(f"wt{i}", [NP, FP], F16))
            for i in range(NSLOT)
        ]
        tmp = ctx.enter_context(nc.sbuf_tensor("tmp", [NP, FP], F16))
        out_t = ctx.enter_context(nc.sbuf_tensor("out_t", [NP, FP], F16))
        block = ctx.enter_context(nc.Block())
        in_sems = [
            ctx.enter_context(nc.semaphore(f"in_sem{q}")) for q in range(4)
        ]
        # private per-tap weight sems; quartered taps get one per quarter
        wq_sems = {
            (j, q): ctx.enter_context(nc.semaphore(f"w{j}q{q}_sem"))
            for j in range(NQTAP)
            for q in range(4)
        }
        w_sems = {
            j: ctx.enter_context(nc.semaphore(f"w{j}_sem"))
            for j in range(NQTAP, K)
        }
        dve_sem = ctx.enter_context(nc.semaphore("dve_sem"))
        out_sem = ctx.enter_context(nc.semaphore("out_sem"))

        in3 = in_pad[:].rearrange("p (r w) -> p r w", r=PR)
        out3 = out_t[:].rearrange("p (r w) -> p r w", r=HH)
        tmp3 = tmp[:].rearrange("p (r w) -> p r w", r=HH)

        # dve_sem: +1 after the LAST weight-read (final mult) of each tap
        # (slot-reuse gate), then +1 per final-accumulate quarter (out gate).

        @block.sync
        def _(sync):
            # interleave input pieces with tap-0 weight quarters
            for q in range(4):
                r0, r1 = IN_PIECES[q]
                sync.dma_start(
                    out=in_pad[:, r0 * PC : r1 * PC],
                    in_=inp[:, r0 * PC : r1 * PC],
                ).then_inc(in_sems[q], 16)
                sync.dma_start(
                    out=wt[0][:, q * QF : (q + 1) * QF],
                    in_=wts[TAPS[0], :, q * QF : (q + 1) * QF],
                ).then_inc(wq_sems[(0, q)], 16)
            for j in range(1, NQTAP):
                for q in range(4):
                    sync.dma_start(
                        out=wt[j][:, q * QF : (q + 1) * QF],
                        in_=wts[TAPS[j], :, q * QF : (q + 1) * QF],
                    ).then_inc(wq_sems[(j, q)], 16)
            for j in range(NQTAP, K):
                if j >= NSLOT:
                    sync.wait_ge(dve_sem, j - NSLOT + 1)
                sync.dma_start(out=wt[j % NSLOT][:], in_=wts[TAPS[j]]).then_inc(
                    w_sems[j], 16
                )
            for q in range(4):
                sync.wait_ge(dve_sem, K + q + 1)
                sync.dma_start(
                    out=out[:, q * QF : (q + 1) * QF],
                    in_=out_t[:, q * QF : (q + 1) * QF],
                ).then_inc(out_sem, 16)
            sync.wait_ge(out_sem, 64)

        @block.vector
        def _(vector):
            for j in range(K):
                k = TAPS[j]
                dh, dw = k // 3, k % 3
                wt3 = wt[j % NSLOT][:].rearrange("p (r w) -> p r w", r=HH)
                if j < NQTAP:
                    # quarter-granular: mult (and for j>0 accumulate) per 16 rows
                    for q in range(4):
                        if j == 0:
                            vector.wait_ge(in_sems[q], 16)
                        vector.wait_ge(wq_sems[(j, q)], 16)
                        r = q * QR
                        i0 = in3[:, r + dh : r + dh + QR, dw : dw + W]
                        if j == 0:
                            mm = vector.tensor_tensor(
                                out=out3[:, r : r + QR],
                                in0=i0,
                                in1=wt3[:, r : r + QR],
                                op=mybir.AluOpType.mult,
                            )
                        else:
                            mm = vector.tensor_tensor(
                                out=tmp3[:, r : r + QR],
                                in0=i0,
                                in1=wt3[:, r : r + QR],
                                op=mybir.AluOpType.mult,
                            )
                        if q == 3:
                            mm.then_inc(dve_sem, 1)
                        if j > 0:
                            vector.tensor_tensor(
                                out=out3[:, r : r + QR],
                                in0=out3[:, r : r + QR],
                                in1=tmp3[:, r : r + QR],
                                op=mybir.AluOpType.add,
                            )
                    continue
                vector.wait_ge(w_sems[j], 16)
                vector.tensor_tensor(
                    out=tmp3,
                    in0=in3[:, dh : dh + HH, dw : dw + W],
                    in1=wt3,
                    op=mybir.AluOpType.mult,
                ).then_inc(dve_sem, 1)
                if j == K - 1:
                    # final accumulate in quarters; each releases an out DMA
                    for q in range(4):
                        r = q * QR
                        vector.tensor_tensor(
                            out=out3[:, r : r + QR],
                            in0=out3[:, r : r + QR],
                            in1=tmp3[:, r : r + QR],
                            op=mybir.AluOpType.add,
                        ).then_inc(dve_sem, 1)
                else:
                    vector.tensor_tensor(
                        out=out3, in0=out3, in1=tmp3, op=mybir.AluOpType.add
                    )

    return nc


def _prep_input(x):
    """(64,128,128) f32 -> (128, 66*130) fp16 per-partition padded layout."""
    pad = np.zeros((CI, H + 2, W + 2), dtype=np.float16)
    pad[:, 1 : H + 1, 1 : W + 1] = x.astype(np.float16)
    win = np.stack([pad[:, 0:PR, :], pad[:, HH : HH + PR, :]], axis=0)
    return np.ascontiguousarray(win.reshape(NP, PR * PC))


def _prep_weights(w):
    """(64,9,128,128) f32 -> (9, 128, 64*128) fp16, partition p = half*64 + c."""
    wr = w.astype(np.float16).reshape(CI, K, 2, HH, W).transpose(1, 2, 0, 3, 4)
    return np.ascontiguousarray(wr.reshape(K, NP, FP))


def _unprep_out(o):
    """(128, 64*128) fp16 -> (64,128,128) f32."""
    return np.ascontiguousarray(
        o.astype(np.float32).reshape(2, CI, HH, W).transpose(1, 0, 2, 3).reshape(CI, H, W)
    )


_NC = None


def _get_nc():
    global _NC
    if _NC is None:
        _NC = build_bass()
    return _NC


def make_in_maps(input, weights):
    input = np.asarray(input, dtype=np.float32)
    weights = np.asarray(weights, dtype=np.float32)
    return [
        {"input": _prep_input(input[b]), "weights": _prep_weights(weights[b])}
        for b in range(B)
    ]


def kernel(input, weights):
    nc = _get_nc()
    in_maps = make_in_maps(input, weights)
    res = run_bass_kernel_spmd(nc, in_maps, list(range(B)))
    return np.stack([_unprep_out(res.results[b]["out"]) for b in range(B)], axis=0)


# revision 11
# speedup vs baseline: 1.9726x; 1.0320x over previous
"""Guided channel-wise 3x3 conv (per-pixel weights) on 8 Trainium2 cores.

out[b,c,h,w] = sum_{dh,dw in {-1,0,1}} input[b,c,h+dh,w+dw] * weights[b,c,k(dh,dw),h,w]
with SAME zero padding.  Shapes: input (8,64,128,128) f32,
weights (8,64,9,128,128) f32 -> out (8,64,128,128) f32.

Sharding: pure data parallelism, one batch sample per NeuronCore (B=8 cores).

Per-core layout: 128 SBUF partitions = (half, c) with p = half*64 + c; each
partition holds one 64-row half of one channel plane (input padded 66x130).
Everything on-chip is fp16: halves HBM traffic (memory-bound regime) and
doubles DVE throughput (2x_1p mode needs 2-byte packed operands); rel err
~6e-4 vs the 2e-2 gate.

The DVE does 9 multiplies + 8 accumulates (~75 us of engine time); the
pipeline hides DMA behind it:
  - taps 0-2 are processed in 16-row quarters gated on quarter-granular
    weight DMAs, so compute starts as soon as the first ~1 MB lands and the
    serial fill (input + first taps) is overlapped,
  - taps 3-8 stream whole planes through 4 buffer slots (DMA runs ahead),
  - the final accumulate runs in quarters, each immediately flushed to HBM.

DMA completions are OUT OF ORDER on this hardware (queue packets fan out
over 16 DMA engines), so a single cumulative DMA semaphore is unsound.
Every awaited transfer group gets a private semaphore, and consumers wait
for that semaphore's full count — correct under any completion order.
"""

import numpy as np

from concourse import bass, mybir
from concourse.bass_utils import run_bass_kernel_spmd

B, CI, H, W = 8, 64, 128, 128
K = 9
HH = H // 2  # rows per half-plane (64)
PR = HH + 2  # padded rows per partition (66)
PC = W + 2  # padded cols (130)
NP = 128  # SBUF partitions
FP = HH * W  # free elems per partition of one output half-plane (8192)
QF = FP // 4  # quarter free elems (2048)
QR = HH // 4  # quarter rows (16)

F16 = mybir.dt.float16

TAPS = [4, 0, 1, 2, 3, 5, 6, 7, 8]  # center tap first: it initializes out
NSLOT = 4
NQTAP = 3  # taps processed in quarters (0..NQTAP-1)

# input DMA pieces: padded row ranges (disjoint); piece q covers the rows
# needed by quarter q of any tap (16q+dh .. 16q+16+dh, dh<=2)
IN_PIECES = [(0, 18), (18, 34), (34, 50), (50, 66)]


def build_bass():
    nc = bass.Bass()
    inp = nc.declare_dram_parameter("input", [NP, PR * PC], F16, isOutput=False)
    wts = nc.declare_dram_parameter("weights", [K, NP, FP], F16, isOutput=False)
    out = nc.declare_dram_parameter("out", [NP, FP], F16, isOutput=True)

    from contextlib import ExitStack

    with ExitStack() as ctx:
        in_pad = ctx.enter_context(nc.sbuf_tensor("in_pad", [NP, PR * PC], F16))
        wt = [
            ctx.enter_context(nc.sbuf_tensor(f"wt{i}", [NP, FP], F16))
            for i in range(NSLOT)
        ]
        tmp = ctx.enter_context(nc.sbuf_tensor("tmp", [NP, FP], F16))
        out_t = ctx.enter_context(nc.sbuf_tensor("out_t", [NP, FP], F16))
        block = ctx.enter_context(nc.Block())
        in_sems = [
            ctx.enter_context(nc.semaphore(f"in_sem{q}")) for q in range(4)
        ]
        # private per-tap weight sems; quartered taps get one per quarter
        wq_sems = {
            (j, q): ctx.enter_context(nc.semaphore(f"w{j}q{q}_sem"))
            for j in range(NQTAP)
            for q in range(4)
        }
        w_sems = {
            j: ctx.enter_context(nc.semaphore(f"w{j}_sem"))
            for j in range(NQTAP, K)
        }
        dve_sem = ctx.enter_context(nc.semaphore("dve_sem"))
        out_sem = ctx.enter_context(nc.semaphore("out_sem"))

        in3 = in_pad[:].rearrange("p (r w) -> p r w", r=PR)
        out3 = out_t[:].rearrange("p (r w) -> p r w", r=HH)
        tmp3 = tmp[:].rearrange("p (r w) -> p r w", r=HH)

        # dve_sem: +1 after the LAST weight-read (final mult) of each tap
        # (slot-reuse gate), then +1 per final-accumulate quarter (out gate).

        @block.sync
        def _(sync):
            # interleave input pieces with tap-0 weight quarters
            for q in range(4):
                r0, r1 = IN_PIECES[q]
                sync.dma_start(
                    out=in_pad[:, r0 * PC : r1 * PC],
                    in_=inp[:, r0 * PC : r1 * PC],
                ).then_inc(in_sems[q], 16)
                sync.dma_start(
                    out=wt[0][:, q * QF : (q + 1) * QF],
                    in_=wts[TAPS[0], :, q * QF : (q + 1) * QF],
                ).then_inc(wq_sems[(0, q)], 16)
            for j in range(1, NQTAP):
                for q in range(4):
                    sync.dma_start(
                        out=wt[j][:, q * QF : (q + 1) * QF],
                        in_=wts[TAPS[j], :, q * QF : (q + 1) * QF],
                    ).then_inc(wq_sems[(j, q)], 16)
            for j in range(NQTAP, K):
                if j >= NSLOT:
                    sync.wait_ge(dve_sem, j - NSLOT + 1)
                sync.dma_start(out=wt[j % NSLOT][:], in_=wts[TAPS[j]]).then_inc(
                    w_sems[j], 16
                )
            for q in range(8):
                sync.wait_ge(dve_sem, K + q + 1)
                sync.dma_start(
                    out=out[:, q * (QF // 2) : (q + 1) * (QF // 2)],
                    in_=out_t[:, q * (QF // 2) : (q + 1) * (QF // 2)],
                ).then_inc(out_sem, 16)
            sync.wait_ge(out_sem, 128)

        @block.vector
        def _(vector):
            for j in range(K):
                k = TAPS[j]
                dh, dw = k // 3, k % 3
                wt3 = wt[j % NSLOT][:].rearrange("p (r w) -> p r w", r=HH)
                if j < NQTAP:
                    # quarter-granular: mult (and for j>0 accumulate) per 16 rows
                    for q in range(4):
                        if j == 0:
                            vector.wait_ge(in_sems[q], 16)
                        vector.wait_ge(wq_sems[(j, q)], 16)
                        r = q * QR
                        i0 = in3[:, r + dh : r + dh + QR, dw : dw + W]
                        if j == 0:
                            mm = vector.tensor_tensor(
                                out=out3[:, r : r + QR],
                                in0=i0,
                                in1=wt3[:, r : r + QR],
                                op=mybir.AluOpType.mult,
                            )
                        else:
                            mm = vector.tensor_tensor(
                                out=tmp3[:, r : r + QR],
                                in0=i0,
                                in1=wt3[:, r : r + QR],
                                op=mybir.AluOpType.mult,
                            )
                        if q == 3:
                            mm.then_inc(dve_sem, 1)
                        if j > 0:
                            vector.tensor_tensor(
                                out=out3[:, r : r + QR],
                                in0=out3[:, r : r + QR],
                                in1=tmp3[:, r : r + QR],
                                op=mybir.AluOpType.add,
                            )
                    continue
                vector.wait_ge(w_sems[j], 16)
                vector.tensor_tensor(
                    out=tmp3,
                    in0=in3[:, dh : dh + HH, dw : dw + W],
                    in1=wt3,
                    op=mybir.AluOpType.mult,
                ).then_inc(dve_sem, 1)
                if j == K - 1:
                    # final accumulate in eighths; each releases an out DMA
                    for q in range(8):
                        r = q * (QR // 2)
                        vector.tensor_tensor(
                            out=out3[:, r : r + QR // 2],
                            in0=out3[:, r : r + QR // 2],
                            in1=tmp3[:, r : r + QR // 2],
                            op=mybir.AluOpType.add,
                        ).then_inc(dve_sem, 1)
                else:
                    vector.tensor_tensor(
                        out=out3, in0=out3, in1=tmp3, op=mybir.AluOpType.add
                    )

    return nc


def _prep_input(x):
    """(64,128,128) f32 -> (128, 66*130) fp16 per-partition padded layout."""
    pad = np.zeros((CI, H + 2, W + 2), dtype=np.float16)
    pad[:, 1 : H + 1, 1 : W + 1] = x.astype(np.float16)
    win = np.stack([pad[:, 0:PR, :], pad[:, HH : HH + PR, :]], axis=0)
    return np.ascontiguousarray(win.reshape(NP, PR * PC))


def _prep_weights(w):
    """(64,9,128,128) f32 -> (9, 128, 64*128) fp16, partition p = half*64 + c."""
    wr = w.astype(np.float16).reshape(CI, K, 2, HH, W).transpose(1, 2, 0, 3, 4)
    return np.ascontiguousarray(wr.reshape(K, NP, FP))


def _unprep_out(o):
    """(128, 64*128) fp16 -> (64,128,128) f32."""
    return np.ascontiguousarray(
        o.astype(np.float32).reshape(2, CI, HH, W).transpose(1, 0, 2, 3).reshape(CI, H, W)
    )


_NC = None


def _get_nc():
    global _NC
    if _NC is None:
        _NC = build_bass()
    return _NC


def make_in_maps(input, weights):
    input = np.asarray(input, dtype=np.float32)
    weights = np.asarray(weights, dtype=np.float32)
    return [
        {"input": _prep_input(input[b]), "weights": _prep_weights(weights[b])}
        for b in range(B)
    ]


def kernel(input, weights):
    nc = _get_nc()
    in_maps = make_in_maps(input, weights)
    res = run_bass_kernel_spmd(nc, in_maps, list(range(B)))
    return np.stack([_unprep_out(res.results[b]["out"]) for b in range(B)], axis=0)
